# revision 10
# baseline (speedup 1.0000x reference)
"""RNN-T joint network kernel for 8 Trainium2 NeuronCores.

out[b,t,u,:] = W2 @ tanh(W1e @ enc[b,t] + W1d @ dec[b,u] + b1) + b2

Shapes: B=4, T=200, U=100, D=512, H=1024, O=512 (fp32 in/out).

Sharding: (b, t-half) per core — core c handles b=c//2, t's
[100*(c%2), 100*(c%2)+100). Each core computes 10000 output rows;
phase 1 (ench/dech) is 6400 PE cycles (vs 16000 for t-sharding,
which replicates dech for all 4 b's on every core).

Precision: phase 1 and 6 of 8 phase-2 h-chunks are bf16; h-chunks
6,7 run as one fp8(e4m3) DoubleRow matmul per output block (2 MACs/
cycle; measured: the DR matmul costs the same issue time as ONE bf16
matmul, halving those chunks' PE time). Measured rel_fro 1.75e-2 vs
the 2e-2 budget (bf16 alone 3.8e-3; full fp8 3.4e-2). W2 is
pre-scaled x32 so its e4m3 values sit in the normal range; the host
divides the output back.

DMA facts measured on HW: a dma_start costs ~600-790ns of ISSUING
ENGINE time (plus ring-credit waits), and the two HWDGE rings are
the Sync and ACT engines. v2 put 11 input DMAs + half the output
DMAs on ACT, which saturated it (89% busy) and delayed phase-2 start
to 21.5us. Now: ACT's ring carries only 6 input pieces (interleaved
with phase-1 emission so identities aren't head-of-line blocked);
ALL output DMA rides the Sync ring (PSUM cannot be a DMA source, so
the psum copies stay on engines: 1 ACT + 3 DVE).

Inputs are split into 2-k-chunk pieces so the PE can start phase 1
as soon as piece 0 lands; 6 dummy 400-col matmuls after phase 1 keep
the PE HAM activity monitor busy so phase 2 starts at the warm 2.4
GHz clock instead of 1.2.

Phase 2 is software-pipelined by two chunks (three near the tail):
mms(i), build(i+2..), copies(i) — the in-order DVE/ACT queues always
hold ready build work ahead of the PE-dependent psum copies. Chunk
sizes [1,2,3,4] + [5]*16 + [4,3,2,1] (rows = 100*t): small at the
start so the first builds finish while input DMA completes, tapered
at the end so the final build chain isn't exposed.

Engine split per 500-row chunk (PE 5.94us): DVE adds k0..4 + the
k6,7 fp8 pre-add + 3 psum copies (~5.3us); GpSimd adds k5 (~3.1us);
ACT tanhs A/B/fp8 + 1 psum copy (~4.8us); Sync ring 4 output DMAs.
"""

from contextlib import ExitStack

import ml_dtypes
import numpy as np

import concourse.bacc as bacc
import concourse.bass as bass
import concourse.mybir as mybir
import concourse.tile as tile
from concourse.bass_utils import run_bass_kernel_spmd

F32 = mybir.dt.float32
BF16 = mybir.dt.bfloat16
F8E4 = mybir.dt.float8e4

B, T, U, D, H, O = 4, 200, 100, 512, 1024, 512
NCORES = 8
TLOC = 100                    # t's per core (half of one b's 200)
ROWS = TLOC * U               # 10000 output rows per core
DK = D // 128                 # 4 contraction chunks for phase 1
HK = H // 128                 # 8 h chunks
NBF = 6                       # bf16 h-chunks (k0..5): 3 on DVE, 3 GpSimd
NA = 3                        # DVE bf16 chunks k0..2
NGP = 3                       # GpSimd bf16 chunks k3..5
NSLOT = 24                    # padded 500-row chunk slots in the out layout
CH = 500                      # max rows per phase-2 chunk
SCALE = 32.0                  # W2 pre-scale so e4m3 values are normal
ENC_W = DK * TLOC             # 400
OCB = O // 128                # 4 output blocks

_CACHE = {}


def _chunks():
    sizes = [1, 2, 3, 4] + [5] * 16 + [4, 3, 2, 1]
    assert sum(sizes) == TLOC
    out, t0 = [], 0
    for tch in sizes:
        out.append((t0, tch))
        t0 += tch
    return out


def _build():
    nc = bacc.Bacc("TRN2", target_bir_lowering=False, debug=False,
                   num_devices=NCORES)
    inS = nc.dram_tensor("inS", [128, ENC_W + HK * 512], BF16,
                         kind="ExternalInput")    # encT | w1e k-major
    inC = nc.dram_tensor("inC", [128, ENC_W + HK * 512], BF16,
                         kind="ExternalInput")    # decT | w1d k-major
    w2b = nc.dram_tensor("w2b", [128, OCB * NBF * 128], BF16,
                         kind="ExternalInput")    # oc-major bf16 W2*32
    w2f = nc.dram_tensor("w2f", [128, OCB * 2 * 128], F8E4,
                         kind="ExternalInput")    # oc-major e4m3 W2*32, k6/7
    b1r = nc.dram_tensor("b1r", [128, HK], F32, kind="ExternalInput")
    # chunk-major, 500-padded slots: row block (oc*NSLOT + slot)*128 holds
    # [128, rows] of output block oc for chunk `slot` — contiguous writes
    out = nc.dram_tensor("out", [OCB * NSLOT * 128, CH], BF16,
                         kind="ExternalOutput")

    with tile.TileContext(nc) as tc, ExitStack() as ctx:
        consts = ctx.enter_context(tc.tile_pool(name="consts", bufs=1))
        spoolA = ctx.enter_context(tc.tile_pool(name="spoolA", bufs=5))
        spoolB = ctx.enter_context(tc.tile_pool(name="spoolB", bufs=5))
        ppool8 = ctx.enter_context(tc.tile_pool(name="ppool8", bufs=5))
        spool8 = ctx.enter_context(tc.tile_pool(name="spool8", bufs=5))
        opool = ctx.enter_context(tc.tile_pool(name="opool", bufs=8))
        psB = ctx.enter_context(tc.tile_pool(name="psB", bufs=8, space="PSUM"))

        b1_s = consts.tile([128, HK], F32)
        inS_s = [consts.tile([128, 1424 if g == 0 else 1024], BF16,
                             name=f"inS{g}") for g in range(4)]
        inC_s = [consts.tile([128, 1424 if g == 0 else 1024], BF16,
                             name=f"inC{g}") for g in range(4)]
        w2b_s = [consts.tile([128, NBF * 128], BF16, name=f"w2b{oc}")
                 for oc in range(OCB)]
        w2f_s = consts.tile([128, OCB * 2 * 128], F8E4)
        ench_s = consts.tile([128, HK * TLOC], BF16)
        dech_s = consts.tile([128, HK * U], BF16)

        def w1_blk(side, k, dk):
            t = (inS_s if side == "e" else inC_s)[k // 2]
            off = (0 if k // 2 else ENC_W) + (k % 2) * 512
            return t[:, off + dk * 128:off + (dk + 1) * 128]

        encT = inS_s[0][:, :ENC_W]
        decT = inC_s[0][:, :ENC_W]

        def p1(side, k):
            n = TLOC if side == "e" else U
            src = encT if side == "e" else decT
            p = psB.tile([128, 512], F32, tag="psB", name="p1")[:, :n]
            for dk in range(DK):
                nc.tensor.matmul(
                    p[:], lhsT=w1_blk(side, k, dk),
                    rhs=src[:, dk * n:(dk + 1) * n],
                    start=(dk == 0), stop=(dk == DK - 1))
            if side == "e":
                nc.scalar.activation(
                    ench_s[:, k * TLOC:(k + 1) * TLOC], p[:],
                    mybir.ActivationFunctionType.Identity,
                    bias=b1_s[:, k:k + 1])
            else:
                nc.vector.tensor_copy(dech_s[:, k * U:(k + 1) * U], p[:])

        # ---- input DMA, interleaved with phase-1 emission ----
        nc.sync.dma_start(b1_s[:], b1r[:])
        W = ENC_W + 1024
        nc.sync.dma_start(inS_s[0][:], inS[:, :W])
        nc.scalar.dma_start(inC_s[0][:], inC[:, :W])
        nc.sync.dma_start(inS_s[1][:], inS[:, W:W + 1024])
        nc.scalar.dma_start(inC_s[1][:], inC[:, W:W + 1024])
        p1("e", 0); p1("d", 0); p1("e", 1); p1("d", 1)
        nc.sync.dma_start(inS_s[2][:], inS[:, W + 1024:W + 2048])
        nc.scalar.dma_start(inC_s[2][:], inC[:, W + 1024:W + 2048])
        p1("e", 2); p1("d", 2); p1("e", 3); p1("d", 3)
        nc.sync.dma_start(inS_s[3][:], inS[:, W + 2048:W + 3072])
        nc.scalar.dma_start(inC_s[3][:], inC[:, W + 2048:W + 3072])
        p1("e", 4); p1("d", 4); p1("e", 5); p1("d", 5)
        wbw = NBF * 128
        nc.sync.dma_start(w2b_s[0][:], w2b[:, :wbw])
        nc.scalar.dma_start(w2b_s[1][:], w2b[:, wbw:2 * wbw])
        p1("e", 6); p1("d", 6); p1("e", 7); p1("d", 7)
        nc.sync.dma_start(w2b_s[2][:], w2b[:, 2 * wbw:3 * wbw])
        nc.scalar.dma_start(w2f_s[:], w2f[:])
        nc.sync.dma_start(w2b_s[3][:], w2b[:, 3 * wbw:])

        # keep the PE HAM window busy so phase 2 starts at 2.4 GHz
        pdum = psB.tile([128, 512], F32, tag="psB", name="pdum")[:, :ENC_W]
        for _ in range(6):
            nc.tensor.matmul(pdum[:], lhsT=w1_blk("e", 0, 0),
                             rhs=encT[:, :ENC_W], start=True, stop=True)

        # ---- phase 2 ----
        chunks = _chunks()
        n_ch = len(chunks)
        sA_t = [None] * n_ch
        sB_t = [None] * n_ch
        s8_t = [None] * n_ch
        ps_t = [None] * n_ch

        def bcast_add(eng, outap, k0, nk, t0c, tch):
            dech_ap = dech_s[:, k0 * U:(k0 + nk) * U].rearrange(
                "p (k u) -> p k u", k=nk).rearrange(
                "p k (a u) -> p k a u", a=1)
            ench_ap = ench_s[:, k0 * TLOC:(k0 + nk) * TLOC].rearrange(
                "p (k t) -> p k t", k=nk)[:, :, t0c:t0c + tch].rearrange(
                "p k (t a) -> p k t a", a=1)
            bc_d, bc_e = bass.broadcast_tensor_aps(dech_ap, ench_ap)
            eng.tensor_tensor(outap, bc_d, bc_e, mybir.AluOpType.add)

        def build(i):
            t0c, tch = chunks[i]
            rows_c = tch * U
            sA = spoolA.tile([128, NA * CH], BF16, tag="sA", name="sA")
            sB = spoolB.tile([128, NGP * CH], BF16, tag="sB", name="sB")
            p8 = ppool8.tile([128, 2 * CH], BF16, tag="p8", name="p8")
            s8 = spool8.tile([128, 2 * 512], F8E4, tag="s8", name="s8")
            sA_t[i], sB_t[i], s8_t[i] = sA, sB, s8
            TANH = mybir.ActivationFunctionType.Tanh
            bcast_add(nc.vector,
                      sA[:, :NA * rows_c].rearrange(
                          "p (k t u) -> p k t u", k=NA, t=tch),
                      0, NA, t0c, tch)
            nc.scalar.activation(sA[:, :NA * rows_c], sA[:, :NA * rows_c],
                                 TANH)
            bcast_add(nc.gpsimd,
                      sB[:, :NGP * rows_c].rearrange(
                          "p (k t u) -> p k t u", k=NGP, t=tch),
                      NA, NGP, t0c, tch)
            nc.scalar.activation(sB[:, :NGP * rows_c], sB[:, :NGP * rows_c],
                                 TANH)
            bcast_add(nc.vector,
                      p8[:, :2 * rows_c].rearrange(
                          "p (k t u) -> p k t u", k=2, t=tch),
                      NBF, 2, t0c, tch)
            s8_ap = s8[:].rearrange("p (j c) -> p j c", j=2)[:, :, :rows_c]
            p8_ap = p8[:, :2 * rows_c].rearrange("p (j c) -> p j c", j=2)
            nc.scalar.activation(s8_ap, p8_ap, TANH)

        def mms(i):
            t0c, tch = chunks[i]
            rows_c = tch * U
            sA, sB, s8 = sA_t[i], sB_t[i], s8_t[i]
            ps = []
            for oc in range(OCB):
                p = psB.tile([128, 512], F32, tag="psB",
                             name="p")[:, :rows_c]
                ps.append(p)
                for k in range(NA):
                    nc.tensor.matmul(
                        p[:], lhsT=w2b_s[oc][:, k * 128:(k + 1) * 128],
                        rhs=sA[:, k * rows_c:(k + 1) * rows_c],
                        start=(k == 0), stop=False)
                for k in range(NGP):
                    nc.tensor.matmul(
                        p[:],
                        lhsT=w2b_s[oc][:, (NA + k) * 128:(NA + k + 1) * 128],
                        rhs=sB[:, k * rows_c:(k + 1) * rows_c],
                        start=False, stop=False)
                nc.tensor.matmul(
                    p[:],
                    lhsT=w2f_s[:, oc * 256:(oc + 1) * 256].rearrange(
                        "p (j f) -> p j f", j=2),
                    rhs=s8[:].rearrange("p (j c) -> p j c", j=2)[:, :, :rows_c],
                    start=False, stop=True,
                    perf_mode=mybir.MatmulPerfMode.DoubleRow)
            ps_t[i] = ps

        def copies(i):
            t0c, tch = chunks[i]
            rows_c = tch * U
            ps = ps_t[i]
            for pair, ring in ((0, nc.sync), (1, nc.scalar)):
                ot = opool.tile([128, 2 * CH], BF16, tag="ot", name="ot")
                for j in range(2):
                    nc.vector.tensor_copy(
                        ot[:, j * CH:j * CH + rows_c], ps[2 * pair + j][:])
                src_ap = ot[:].rearrange(
                    "p (j c) -> p j c", j=2)[:, :, :rows_c]
                dst = out[:].rearrange(
                    "(oc s p) c -> oc s p c", oc=OCB, s=NSLOT)[
                    2 * pair:2 * pair + 2, i, :, :rows_c].rearrange(
                    "oc p c -> p oc c")
                ring.dma_start(dst, src_ap)

        build(0)
        build(1)
        built = 2
        for i in range(n_ch):
            mms(i)
            depth = 2 if i < n_ch - 8 else 3
            while built < min(i + depth, n_ch):
                build(built)
                built += 1
            copies(i)
    nc.compile()
    return nc


def _chunk128(a):
    # [n*128, w] -> [128, n*w]: partition p holds row k*128+p of chunk k
    n = a.shape[0] // 128
    return np.ascontiguousarray(
        a.reshape(n, 128, a.shape[1]).transpose(1, 0, 2).reshape(128, -1))


def _bf16(a):
    return np.ascontiguousarray(a).astype(ml_dtypes.bfloat16)


def _kmajor(w1T):
    # [128, dk-major (DK x H)] -> [128, k-major (HK x DK x 128)]
    return np.ascontiguousarray(
        w1T.reshape(128, DK, HK, 128).transpose(0, 2, 1, 3).reshape(128, -1))


def kernel(enc_state, dec_state, W1, b1, W2, b2, _trace=False):
    enc_state = np.ascontiguousarray(enc_state, dtype=np.float32)
    dec_state = np.ascontiguousarray(dec_state, dtype=np.float32)
    W1 = np.asarray(W1, dtype=np.float32)
    b1 = np.asarray(b1, dtype=np.float32)
    W2 = np.asarray(W2, dtype=np.float32)
    b2 = np.asarray(b2, dtype=np.float32)

    if "nc" not in _CACHE:
        _CACHE["nc"] = _build()
    nc = _CACHE["nc"]

    w1e_km = _bf16(_kmajor(_chunk128(W1[:, :D].T)))
    w1d_km = _bf16(_kmajor(_chunk128(W1[:, D:].T)))
    b1r = np.ascontiguousarray(b1.reshape(HK, 128).T)

    # W2*32, chunked [128, hk, o]: element [p, hk, o] = 32*W2[o, hk*128+p]
    w2c = _chunk128((W2.T * SCALE).astype(np.float32)).reshape(128, HK, O)
    w2b = _bf16(np.ascontiguousarray(
        w2c[:, :NBF, :].reshape(128, NBF, OCB, 128).transpose(0, 2, 1, 3)
        .reshape(128, -1)))
    w2f = np.ascontiguousarray(
        w2c[:, NBF:, :].reshape(128, 2, OCB, 128).transpose(0, 2, 1, 3)
        .reshape(128, -1)).astype(ml_dtypes.float8_e4m3)

    decT = {}
    for b in range(B):
        decT[b] = _bf16(_chunk128(dec_state[b].T))          # [128, DK*U]

    in_maps = []
    for c in range(NCORES):
        b, th = c // 2, c % 2
        enc_c = enc_state[b, th * TLOC:(th + 1) * TLOC]     # [100, 512]
        encT = _bf16(_chunk128(enc_c.T))                    # [128, DK*100]
        in_maps.append({
            "inS": np.concatenate([encT, w1e_km], axis=1),
            "inC": np.concatenate([decT[b], w1d_km], axis=1),
            "w2b": w2b, "w2f": w2f, "b1r": b1r,
        })

    res = run_bass_kernel_spmd(nc, in_maps, list(range(NCORES)), trace=_trace)
    out = np.empty((B, T, U, O), dtype=np.float32)
    for c in range(NCORES):
        b, th = c // 2, c % 2
        o4 = res.results[c]["out"].reshape(OCB, NSLOT, 128, CH)
        full = np.empty((O, ROWS), dtype=np.float32)
        for i, (t0c, tch) in enumerate(_chunks()):
            full[:, t0c * U:(t0c + tch) * U] = (
                o4[:, i, :, :tch * U].astype(np.float32).reshape(O, -1))
        full /= SCALE
        out[b, th * TLOC:(th + 1) * TLOC] = full.T.reshape(TLOC, U, O)
    out += b2
    if _trace:
        kernel.last_results = res
    return out


# revision 11
# speedup vs baseline: 1.0412x; 1.0412x over previous
"""RNN-T joint network kernel for 8 Trainium2 NeuronCores.

out[b,t,u,:] = W2 @ tanh(W1e @ enc[b,t] + W1d @ dec[b,u] + b1) + b2

Shapes: B=4, T=200, U=100, D=512, H=1024, O=512 (fp32 in/out).

Sharding: (b, t-half) per core — core c handles b=c//2, t's
[100*(c%2), 100*(c%2)+100). Each core computes 10000 output rows;
phase 1 (ench/dech) is 6400 PE cycles (vs 16000 for t-sharding,
which replicates dech for all 4 b's on every core).

Precision: phase 1 and 6 of 8 phase-2 h-chunks are bf16; h-chunks
6,7 run as one fp8(e4m3) DoubleRow matmul per output block (2 MACs/
cycle; measured: the DR matmul costs the same issue time as ONE bf16
matmul, halving those chunks' PE time). Measured rel_fro 1.75e-2 vs
the 2e-2 budget (bf16 alone 3.8e-3; full fp8 3.4e-2). W2 is
pre-scaled x32 so its e4m3 values sit in the normal range; the host
divides the output back.

DMA facts measured on HW: a dma_start costs ~600-790ns of ISSUING
ENGINE time (plus ring-credit waits), and the two HWDGE rings are
the Sync and ACT engines. v2 put 11 input DMAs + half the output
DMAs on ACT, which saturated it (89% busy) and delayed phase-2 start
to 21.5us. Now: ACT's ring carries only 6 input pieces (interleaved
with phase-1 emission so identities aren't head-of-line blocked);
ALL output DMA rides the Sync ring (PSUM cannot be a DMA source, so
the psum copies stay on engines: 1 ACT + 3 DVE).

Inputs are split into 2-k-chunk pieces so the PE can start phase 1
as soon as piece 0 lands; 6 dummy 400-col matmuls after phase 1 keep
the PE HAM activity monitor busy so phase 2 starts at the warm 2.4
GHz clock instead of 1.2.

Phase 2 is software-pipelined by two chunks (three near the tail):
mms(i), build(i+2..), copies(i) — the in-order DVE/ACT queues always
hold ready build work ahead of the PE-dependent psum copies. Chunk
sizes [1,2,3,4] + [5]*16 + [4,3,2,1] (rows = 100*t): small at the
start so the first builds finish while input DMA completes, tapered
at the end so the final build chain isn't exposed.

Engine split per 500-row chunk (PE 5.94us): DVE adds k0..4 + the
k6,7 fp8 pre-add + 3 psum copies (~5.3us); GpSimd adds k5 (~3.1us);
ACT tanhs A/B/fp8 + 1 psum copy (~4.8us); Sync ring 4 output DMAs.
"""

from contextlib import ExitStack

import ml_dtypes
import numpy as np

import concourse.bacc as bacc
import concourse.bass as bass
import concourse.mybir as mybir
import concourse.tile as tile
from concourse.bass_utils import run_bass_kernel_spmd

F32 = mybir.dt.float32
BF16 = mybir.dt.bfloat16
F8E4 = mybir.dt.float8e4

B, T, U, D, H, O = 4, 200, 100, 512, 1024, 512
NCORES = 8
TLOC = 100                    # t's per core (half of one b's 200)
ROWS = TLOC * U               # 10000 output rows per core
DK = D // 128                 # 4 contraction chunks for phase 1
HK = H // 128                 # 8 h chunks
NBF = 6                       # bf16 h-chunks (k0..5): 3 on DVE, 3 GpSimd
NA = 3                        # DVE bf16 chunks k0..2
NGP = 3                       # GpSimd bf16 chunks k3..5
NSLOT = 24                    # padded 500-row chunk slots in the out layout
CH = 500                      # max rows per phase-2 chunk
SCALE = 32.0                  # W2 pre-scale so e4m3 values are normal
ENC_W = DK * TLOC             # 400
OCB = O // 128                # 4 output blocks

_CACHE = {}


def _chunks():
    sizes = [1, 2, 3, 4] + [5] * 16 + [4, 3, 2, 1]
    assert sum(sizes) == TLOC
    out, t0 = [], 0
    for tch in sizes:
        out.append((t0, tch))
        t0 += tch
    return out


def _build():
    nc = bacc.Bacc("TRN2", target_bir_lowering=False, debug=False,
                   num_devices=NCORES)
    inS = nc.dram_tensor("inS", [128, ENC_W + HK * 512], BF16,
                         kind="ExternalInput")    # encT | w1e k-major
    inC = nc.dram_tensor("inC", [128, ENC_W + HK * 512], BF16,
                         kind="ExternalInput")    # decT | w1d k-major
    w2b = nc.dram_tensor("w2b", [128, OCB * NBF * 128], BF16,
                         kind="ExternalInput")    # oc-major bf16 W2*32
    w2f = nc.dram_tensor("w2f", [128, OCB * 2 * 128], F8E4,
                         kind="ExternalInput")    # oc-major e4m3 W2*32, k6/7
    b1r = nc.dram_tensor("b1r", [128, HK], F32, kind="ExternalInput")
    # chunk-major, 500-padded slots: row block (oc*NSLOT + slot)*128 holds
    # [128, rows] of output block oc for chunk `slot` — contiguous writes
    out = nc.dram_tensor("out", [OCB * NSLOT * 128, CH], BF16,
                         kind="ExternalOutput")

    with tile.TileContext(nc) as tc, ExitStack() as ctx:
        consts = ctx.enter_context(tc.tile_pool(name="consts", bufs=1))
        spoolA = ctx.enter_context(tc.tile_pool(name="spoolA", bufs=5))
        spoolB = ctx.enter_context(tc.tile_pool(name="spoolB", bufs=5))
        ppool8 = ctx.enter_context(tc.tile_pool(name="ppool8", bufs=5))
        spool8 = ctx.enter_context(tc.tile_pool(name="spool8", bufs=5))
        opool = ctx.enter_context(tc.tile_pool(name="opool", bufs=8))
        psB = ctx.enter_context(tc.tile_pool(name="psB", bufs=8, space="PSUM"))

        b1_s = consts.tile([128, HK], F32)
        inS_s = [consts.tile([128, 1424 if g == 0 else 1024], BF16,
                             name=f"inS{g}") for g in range(4)]
        inC_s = [consts.tile([128, 1424 if g == 0 else 1024], BF16,
                             name=f"inC{g}") for g in range(4)]
        w2b_s = [consts.tile([128, NBF * 128], BF16, name=f"w2b{oc}")
                 for oc in range(OCB)]
        w2f_s = consts.tile([128, OCB * 2 * 128], F8E4)
        ench_s = consts.tile([128, HK * TLOC], BF16)
        dech_s = consts.tile([128, HK * U], BF16)

        def w1_blk(side, k, dk):
            t = (inS_s if side == "e" else inC_s)[k // 2]
            off = (0 if k // 2 else ENC_W) + (k % 2) * 512
            return t[:, off + dk * 128:off + (dk + 1) * 128]

        encT = inS_s[0][:, :ENC_W]
        decT = inC_s[0][:, :ENC_W]

        def p1(side, k):
            n = TLOC if side == "e" else U
            src = encT if side == "e" else decT
            p = psB.tile([128, 512], F32, tag="psB", name="p1")[:, :n]
            for dk in range(DK):
                nc.tensor.matmul(
                    p[:], lhsT=w1_blk(side, k, dk),
                    rhs=src[:, dk * n:(dk + 1) * n],
                    start=(dk == 0), stop=(dk == DK - 1))
            if side == "e":
                nc.scalar.activation(
                    ench_s[:, k * TLOC:(k + 1) * TLOC], p[:],
                    mybir.ActivationFunctionType.Identity,
                    bias=b1_s[:, k:k + 1])
            else:
                nc.vector.tensor_copy(dech_s[:, k * U:(k + 1) * U], p[:])

        # ---- input DMA, interleaved with phase-1 emission ----
        nc.sync.dma_start(b1_s[:], b1r[:])
        W = ENC_W + 1024
        nc.sync.dma_start(inS_s[0][:], inS[:, :W])
        nc.scalar.dma_start(inC_s[0][:], inC[:, :W])
        nc.sync.dma_start(inS_s[1][:], inS[:, W:W + 1024])
        nc.scalar.dma_start(inC_s[1][:], inC[:, W:W + 1024])
        p1("e", 0); p1("d", 0); p1("e", 1); p1("d", 1)
        nc.sync.dma_start(inS_s[2][:], inS[:, W + 1024:W + 2048])
        nc.scalar.dma_start(inC_s[2][:], inC[:, W + 1024:W + 2048])
        p1("e", 2); p1("d", 2); p1("e", 3); p1("d", 3)
        nc.sync.dma_start(inS_s[3][:], inS[:, W + 2048:W + 3072])
        nc.scalar.dma_start(inC_s[3][:], inC[:, W + 2048:W + 3072])
        p1("e", 4); p1("d", 4); p1("e", 5); p1("d", 5)
        wbw = NBF * 128
        nc.sync.dma_start(w2b_s[0][:], w2b[:, :wbw])
        nc.scalar.dma_start(w2b_s[1][:], w2b[:, wbw:2 * wbw])
        p1("e", 6); p1("d", 6); p1("e", 7); p1("d", 7)
        nc.sync.dma_start(w2b_s[2][:], w2b[:, 2 * wbw:3 * wbw])
        nc.scalar.dma_start(w2f_s[:], w2f[:])
        nc.sync.dma_start(w2b_s[3][:], w2b[:, 3 * wbw:])

        # keep the PE HAM window busy so phase 2 starts at 2.4 GHz
        pdum = psB.tile([128, 512], F32, tag="psB", name="pdum")[:, :ENC_W]
        for _ in range(6):
            nc.tensor.matmul(pdum[:], lhsT=w1_blk("e", 0, 0),
                             rhs=encT[:, :ENC_W], start=True, stop=True)

        # ---- phase 2 ----
        chunks = _chunks()
        n_ch = len(chunks)
        sA_t = [None] * n_ch
        sB_t = [None] * n_ch
        s8_t = [None] * n_ch
        ps_t = [None] * n_ch

        def bcast_add(eng, outap, k0, nk, t0c, tch):
            dech_ap = dech_s[:, k0 * U:(k0 + nk) * U].rearrange(
                "p (k u) -> p k u", k=nk).rearrange(
                "p k (a u) -> p k a u", a=1)
            ench_ap = ench_s[:, k0 * TLOC:(k0 + nk) * TLOC].rearrange(
                "p (k t) -> p k t", k=nk)[:, :, t0c:t0c + tch].rearrange(
                "p k (t a) -> p k t a", a=1)
            bc_d, bc_e = bass.broadcast_tensor_aps(dech_ap, ench_ap)
            eng.tensor_tensor(outap, bc_d, bc_e, mybir.AluOpType.add)

        def build(i):
            t0c, tch = chunks[i]
            rows_c = tch * U
            sA = spoolA.tile([128, NA * CH], BF16, tag="sA", name="sA")
            sB = spoolB.tile([128, NGP * CH], BF16, tag="sB", name="sB")
            p8 = ppool8.tile([128, 2 * CH], BF16, tag="p8", name="p8")
            s8 = spool8.tile([128, 2 * 512], F8E4, tag="s8", name="s8")
            sA_t[i], sB_t[i], s8_t[i] = sA, sB, s8
            TANH = mybir.ActivationFunctionType.Tanh
            bcast_add(nc.vector,
                      sA[:, :NA * rows_c].rearrange(
                          "p (k t u) -> p k t u", k=NA, t=tch),
                      0, NA, t0c, tch)
            nc.scalar.activation(sA[:, :NA * rows_c], sA[:, :NA * rows_c],
                                 TANH)
            bcast_add(nc.gpsimd,
                      sB[:, :NGP * rows_c].rearrange(
                          "p (k t u) -> p k t u", k=NGP, t=tch),
                      NA, NGP, t0c, tch)
            nc.scalar.activation(sB[:, :NGP * rows_c], sB[:, :NGP * rows_c],
                                 TANH)
            bcast_add(nc.vector,
                      p8[:, :2 * rows_c].rearrange(
                          "p (k t u) -> p k t u", k=2, t=tch),
                      NBF, 2, t0c, tch)
            s8_ap = s8[:].rearrange("p (j c) -> p j c", j=2)[:, :, :rows_c]
            p8_ap = p8[:, :2 * rows_c].rearrange("p (j c) -> p j c", j=2)
            nc.scalar.activation(s8_ap, p8_ap, TANH)

        def mms(i):
            t0c, tch = chunks[i]
            rows_c = tch * U
            sA, sB, s8 = sA_t[i], sB_t[i], s8_t[i]
            ps = []
            for oc in range(OCB):
                p = psB.tile([128, 512], F32, tag="psB",
                             name="p")[:, :rows_c]
                ps.append(p)
                for k in range(NA):
                    nc.tensor.matmul(
                        p[:], lhsT=w2b_s[oc][:, k * 128:(k + 1) * 128],
                        rhs=sA[:, k * rows_c:(k + 1) * rows_c],
                        start=(k == 0), stop=False)
                for k in range(NGP):
                    nc.tensor.matmul(
                        p[:],
                        lhsT=w2b_s[oc][:, (NA + k) * 128:(NA + k + 1) * 128],
                        rhs=sB[:, k * rows_c:(k + 1) * rows_c],
                        start=False, stop=False)
                nc.tensor.matmul(
                    p[:],
                    lhsT=w2f_s[:, oc * 256:(oc + 1) * 256].rearrange(
                        "p (j f) -> p j f", j=2),
                    rhs=s8[:].rearrange("p (j c) -> p j c", j=2)[:, :, :rows_c],
                    start=False, stop=True,
                    perf_mode=mybir.MatmulPerfMode.DoubleRow)
            ps_t[i] = ps

        def copies(i):
            t0c, tch = chunks[i]
            rows_c = tch * U
            ps = ps_t[i]
            for pair, ring in ((0, nc.sync), (1, nc.sync)):
                ot = opool.tile([128, 2 * CH], BF16, tag="ot", name="ot")
                for j in range(2):
                    if pair == 0:
                        nc.scalar.activation(
                            ot[:, j * CH:j * CH + rows_c], ps[2 * pair + j][:],
                            mybir.ActivationFunctionType.Copy)
                    else:
                        nc.vector.tensor_copy(
                            ot[:, j * CH:j * CH + rows_c], ps[2 * pair + j][:])
                src_ap = ot[:].rearrange(
                    "p (j c) -> p j c", j=2)[:, :, :rows_c]
                dst = out[:].rearrange(
                    "(oc s p) c -> oc s p c", oc=OCB, s=NSLOT)[
                    2 * pair:2 * pair + 2, i, :, :rows_c].rearrange(
                    "oc p c -> p oc c")
                ring.dma_start(dst, src_ap)

        build(0)
        build(1)
        built = 2
        for i in range(n_ch):
            mms(i)
            depth = 2 if i < n_ch - 8 else 3
            while built < min(i + depth, n_ch):
                build(built)
                built += 1
            copies(i)
    nc.compile()
    return nc


def _chunk128(a):
    # [n*128, w] -> [128, n*w]: partition p holds row k*128+p of chunk k
    n = a.shape[0] // 128
    return np.ascontiguousarray(
        a.reshape(n, 128, a.shape[1]).transpose(1, 0, 2).reshape(128, -1))


def _bf16(a):
    return np.ascontiguousarray(a).astype(ml_dtypes.bfloat16)


def _kmajor(w1T):
    # [128, dk-major (DK x H)] -> [128, k-major (HK x DK x 128)]
    return np.ascontiguousarray(
        w1T.reshape(128, DK, HK, 128).transpose(0, 2, 1, 3).reshape(128, -1))


def kernel(enc_state, dec_state, W1, b1, W2, b2, _trace=False):
    enc_state = np.ascontiguousarray(enc_state, dtype=np.float32)
    dec_state = np.ascontiguousarray(dec_state, dtype=np.float32)
    W1 = np.asarray(W1, dtype=np.float32)
    b1 = np.asarray(b1, dtype=np.float32)
    W2 = np.asarray(W2, dtype=np.float32)
    b2 = np.asarray(b2, dtype=np.float32)

    if "nc" not in _CACHE:
        _CACHE["nc"] = _build()
    nc = _CACHE["nc"]

    w1e_km = _bf16(_kmajor(_chunk128(W1[:, :D].T)))
    w1d_km = _bf16(_kmajor(_chunk128(W1[:, D:].T)))
    b1r = np.ascontiguousarray(b1.reshape(HK, 128).T)

    # W2*32, chunked [128, hk, o]: element [p, hk, o] = 32*W2[o, hk*128+p]
    w2c = _chunk128((W2.T * SCALE).astype(np.float32)).reshape(128, HK, O)
    w2b = _bf16(np.ascontiguousarray(
        w2c[:, :NBF, :].reshape(128, NBF, OCB, 128).transpose(0, 2, 1, 3)
        .reshape(128, -1)))
    w2f = np.ascontiguousarray(
        w2c[:, NBF:, :].reshape(128, 2, OCB, 128).transpose(0, 2, 1, 3)
        .reshape(128, -1)).astype(ml_dtypes.float8_e4m3)

    decT = {}
    for b in range(B):
        decT[b] = _bf16(_chunk128(dec_state[b].T))          # [128, DK*U]

    in_maps = []
    for c in range(NCORES):
        b, th = c // 2, c % 2
        enc_c = enc_state[b, th * TLOC:(th + 1) * TLOC]     # [100, 512]
        encT = _bf16(_chunk128(enc_c.T))                    # [128, DK*100]
        in_maps.append({
            "inS": np.concatenate([encT, w1e_km], axis=1),
            "inC": np.concatenate([decT[b], w1d_km], axis=1),
            "w2b": w2b, "w2f": w2f, "b1r": b1r,
        })

    res = run_bass_kernel_spmd(nc, in_maps, list(range(NCORES)), trace=_trace)
    out = np.empty((B, T, U, O), dtype=np.float32)
    for c in range(NCORES):
        b, th = c // 2, c % 2
        o4 = res.results[c]["out"].reshape(OCB, NSLOT, 128, CH)
        full = np.empty((O, ROWS), dtype=np.float32)
        for i, (t0c, tch) in enumerate(_chunks()):
            full[:, t0c * U:(t0c + tch) * U] = (
                o4[:, i, :, :tch * U].astype(np.float32).reshape(O, -1))
        full /= SCALE
        out[b, th * TLOC:(th + 1) * TLOC] = full.T.reshape(TLOC, U, O)
    out += b2
    if _trace:
        kernel.last_results = res
    return out
